# revision 55
# baseline (speedup 1.0000x reference)
"""Trainium2 Bass kernel for AttentionWithRotaryPosEmb (8 cores, data-parallel).

Strategy
--------
Data-parallel over batch: each of the 8 NeuronCores computes one batch element
end-to-end.  No collectives.

The softmax exp on the ACT engine is the hard floor (64 x [128,1024] PSUM->SBUF
exp tiles ~= 68us); everything else is scheduled around keeping ACT 100% busy:

  1. QKV projection from host-interleaved, chunk-major weight/x layouts so
     the first DMAs carry exactly what the first matmuls need, contiguously.
  2. RoPE + l2-norm(seq axis) per head-pair tile; both q/k scales fold into
     q; t=0 stats ride ACT (idle pre-attention), later tiles DVE mul+reduce.
  3. Attention per HEAD PAIR, software-pipelined per j-block: simA/simB ->
     exp A/B on ACT -> av lagging two j-blocks; the previous pair's av(6/7)
     catch-up and drain ride the next pair's first two (PE-light) cycles.
     PSUM: 2 rotating sim tiles (ppm, 4 banks) + one [128,2,S] av accumulator
     per pair (ppo, 4 banks) = exactly 8 banks.  Softmax denominators come
     free from a ones-column appended to vT.
  4. Softmax normalization: denominator row DMA-reshaped to [128,16], DVE
     reciprocal, DMA back to a [1,S] row, partition-broadcast via K=1
     matmuls into a ring-borrowed PSUM tile, one DVE mul.  For pairs 0-2 the
     whole chain hides under the next pair (norm_mm emitted as a mid-pair
     filler); the last pair's chain is pipelined per head at the tail with
     parks on the then-idle ACT.
  5. Output projection entirely in PSUM at the tail: pair-contraction
     partials kk=0..2 cover the last denominator roundtrip; kk=3 after the
     final normalization; bias split ACT/DVE; bf16 output store (host casts).
  6. HAM management (the PE clock gate defaults to 1.2GHz; ~3.4us of busy
     releases 2.4GHz, one idle window re-throttles): junk matmuls warm the
     gate during the input-DMA head and the rope0 tail, and a per-cycle junk
     "heartbeat" into the soon-overwritten sim tile keeps the attention
     stream dense enough that the gate stays open (throttle_active dropped
     ~87us -> ~25-33us; K_BEAT=0/2 both measured worse than 1).

  Notes: tensor_tensor_reduce crashes at runtime on this hw/runtime combo;
  GPSIMD extended ops unavailable (bedrock image) and its tensor ops run at
  0.42 efficiency (too slow to offload rope); partition starts for engine APs
  must be 0/32/64/96 (no DMA doubling-chain broadcast); fp8 is numerically
  out: exp(10*s) amplifies quantization ~10x.  Exec ~133-138us measured
  (throttle-lottery dependent) vs the 144us session baseline.  Attempted and
  measured WORSE: fillers at cycle 7 (the ring slot aliases the next pair's
  sims), norm muls right after the epilogue DVE burst, split qk evac halves,
  et bufs 8, K_BEAT 0/2, 3 boundary beats.
"""

import os
import sys

import numpy as np

if "/opt/trn_rl_repo" not in sys.path:
    sys.path.insert(0, "/opt/trn_rl_repo")

HEADS = 8
DH = 64
S = 1024
C = 256
HID = 512
ROT = 32
HALF = 16
SCALE = 10.0
N_CORES = 8
N_JUNK = int(os.environ.get("K_JUNK", "7"))  # cold F=512 junk ~580ns each
N_BEAT = int(os.environ.get("K_BEAT", "1"))  # junk heartbeats per sim cycle (0-2)

# rotate-half as a 32-partition-group shuffle: swap the 16-row halves
SHUF_MASK = list(range(16, 32)) + list(range(16))

_CACHE = {}


def _rope_tables():
    """Row-patterned cos/sin tables [128, S] matching the q/k SBUF layout.

    Partition row r holds head 2t + (r>=64), d = r % 64 of tile t.  Rows d in
    [0,16) get cos(i*invf[d]) / -sin(i*invf[d]); rows d in [16,32) get
    cos(i*invf[d-16]) / +sin(i*invf[d-16]); rows d >= 32 get cos=1, sin=0.
    The sign of sin encodes rotate_half.
    """
    inv = (
        1.0 / (np.float32(10000.0) ** (np.arange(0, ROT, 2, dtype=np.float32) / np.float32(ROT)))
    ).astype(np.float32)
    ang = (np.arange(S, dtype=np.float32)[None, :] * inv[:, None]).astype(np.float32)
    cos16 = np.cos(ang).astype(np.float32)  # [16, S]
    sin16 = np.sin(ang).astype(np.float32)
    cosT = np.ones((128, S), np.float32)
    sinT = np.zeros((128, S), np.float32)
    for r in range(128):
        d = r % 64
        if d < HALF:
            cosT[r] = cos16[d]
            sinT[r] = -sin16[d]
        elif d < ROT:
            cosT[r] = cos16[d - HALF]
            sinT[r] = sin16[d - HALF]
    return cosT, sinT


def _emit(ctx, tc, aps):
    import concourse.bass as bass  # noqa: F401
    from concourse import mybir

    f32 = mybir.dt.float32
    bf16 = mybir.dt.bfloat16
    AF = mybir.ActivationFunctionType
    nc = tc.nc
    x_d, wqk_d, wv_d, wo_d, bo_d, cos_d, sin_d, out_d = aps

    singles = ctx.enter_context(tc.tile_pool(name="singles", bufs=1))
    wk = ctx.enter_context(tc.tile_pool(name="wk", bufs=3))
    ppm = ctx.enter_context(tc.tile_pool(name="ppm", bufs=2, space="PSUM"))
    ppo = ctx.enter_context(tc.tile_pool(name="ppo", bufs=1, space="PSUM"))

    # ---- persistent SBUF ----
    sb_x = singles.tile([128, 2, S], bf16)
    sb_wqk = singles.tile([128, 2, 2 * HID], bf16)  # blocks q0,k0,q1,k1,...
    sb_wv = singles.tile([128, 2, HID], bf16)
    sb_wo = singles.tile([128, 4, C], bf16)
    sb_bo = singles.tile([128, 2], f32)
    sb_cos = singles.tile([128, S], bf16)
    sb_sin = singles.tile([128, S], bf16)
    sb_q = singles.tile([128, 4, S], bf16)   # raw q (roped in place)
    sb_k = singles.tile([128, 4, S], bf16)   # raw k (k*cos staging in-place)
    sb_qb = singles.tile([128, 4, S], bf16)  # roped+scaled q, sim operand
    sb_kb = singles.tile([128, 4, S], bf16)  # roped k, sim operand
    sb_vx = singles.tile([128, 8, HEADS * (DH + 1)], bf16)
    sb_or = singles.tile([128, 4, S], bf16)  # attention out, unnormalized
    sb_o = singles.tile([128, 4, S], bf16)   # attention out, proj operand
    sb_stat = singles.tile([128, 24], f32)
    den_sh = singles.tile([128, 4, 2, 8], f32)   # partition-spread denominators
    rcp_sh = singles.tile([128, 4, 2, 8], bf16)  # reciprocals
    ones_col = singles.tile([1, 64], bf16)       # lhsT for the rcp broadcast
    junk_w = singles.tile([128, 512], bf16)      # HAM warm-up operand
    ot = singles.tile([128, 2, S], bf16)         # output staging

    # ---- memsets (ready before anything needs them) ----
    nc.vector.memset(junk_w[:, :], 0.125)
    vx4 = sb_vx.rearrange("p j (h e) -> p j h e", e=DH + 1)
    nc.vector.memset(vx4[:, :, :, DH : DH + 1], 1.0)
    nc.vector.memset(ones_col[:, :], 1.0)

    # ---- input DMAs, priority order (Sync issues serialize ~0.65us each);
    # x and wqk are shipped chunk-major so each DMA reads contiguous DRAM ----
    nc.sync.dma_start(out=sb_wqk[:, :, 0:256], in_=wqk_d[0, :, :, :])
    nc.sync.dma_start(out=sb_x[:, :, 0:512], in_=x_d[0, :, :, :])
    nc.sync.dma_start(out=sb_x[:, :, 512:1024], in_=x_d[1, :, :, :])
    nc.sync.dma_start(out=sb_wv[:, :, :], in_=wv_d[:, :, :])
    nc.sync.dma_start(out=sb_cos[:, :], in_=cos_d[:, :])
    nc.sync.dma_start(out=sb_sin[:, :], in_=sin_d[:, :])
    for c in range(1, 4):
        nc.sync.dma_start(
            out=sb_wqk[:, :, 256 * c : 256 * (c + 1)], in_=wqk_d[c, :, :, :]
        )
    nc.sync.dma_start(out=sb_wo[:, :, :], in_=wo_d[:, :, :])
    nc.sync.dma_start(out=sb_bo[:, :], in_=bo_d[:, :])

    def junk_run(n, name):
        """HAM warm-up / keep-warm: F=512 junk matmuls (~213ns cold each)."""
        if n <= 0:
            return
        jp = ppm.tile([128, S], f32, tag="mm", name=name)
        for i in range(n):
            nc.tensor.matmul(
                jp[:, 0:512], lhsT=junk_w[:, 0:128], rhs=junk_w[:, :],
                start=True, stop=True, skip_group_check=True,
            )

    # junk while the input DMAs land: PE busy >=3.4us releases the clock gate
    junk_run(N_JUNK, "junk0")

    # ---- qkv projection blocks ----
    qk_ps = {}

    def qk_mms(b, nns=(0, 1)):
        """o-block b (q tile b//2 if even, k tile b//2 if odd)."""
        if b not in qk_ps:
            qk_ps[b] = ppm.tile([128, S], f32, tag="mm", name=f"qk{b}")
        ps = qk_ps[b]
        for nn in nns:
            for kk in range(2):
                nc.tensor.matmul(
                    ps[:, nn * 512 : (nn + 1) * 512],
                    lhsT=sb_wqk[:, kk, b * 128 : (b + 1) * 128],
                    rhs=sb_x[:, kk, nn * 512 : (nn + 1) * 512],
                    start=(kk == 0),
                    stop=(kk == 1),
                )

    def qk_evac(b, engine, split=False):
        dst = (sb_q if b % 2 == 0 else sb_k)[:, b // 2, :]
        ps = qk_ps.pop(b)
        op = nc.scalar.copy if engine == "act" else nc.vector.tensor_copy
        if split:
            # per-nn halves so the first half lands right after its matmuls
            for nn in range(2):
                op(out=dst[:, nn * 512 : (nn + 1) * 512],
                   in_=ps[:, nn * 512 : (nn + 1) * 512])
        else:
            op(out=dst, in_=ps[:, :])

    def v_block(jb, evac="dve"):
        """vT s-block: [s_jb, hid] straight from matmul, strided into vx ext."""
        psv = ppm.tile([128, S], f32, tag="mm", name=f"v{jb}")
        for kk in range(2):
            nc.tensor.matmul(
                psv[:, 0:512],
                lhsT=sb_x[:, kk, jb * 128 : (jb + 1) * 128],
                rhs=sb_wv[:, kk, :],
                start=(kk == 0),
                stop=(kk == 1),
            )
        dst = sb_vx[:, jb, :].rearrange("p (h e) -> p h e", e=DH + 1)[:, :, 0:DH]
        src = psv[:, 0:512].rearrange("p (h d) -> p h d", d=DH)
        if evac == "act":
            nc.scalar.copy(out=dst, in_=src)
        else:
            nc.vector.tensor_copy(out=dst, in_=src)

    def rope_norm(t):
        """RoPE + l2-norm(seq) stats for tile t; fold both scales into q.

        t=0 uses ACT Square+accum for the stats (ACT is idle pre-attention);
        later tiles run mul+reduce on DVE (ACT is exp-saturated).
        """
        for src3, col in ((sb_q, 0), (sb_k, 4)):
            cur = src3[:, t, :]
            rot = wk.tile([128, S], bf16, tag="rot", name=f"rot{t}_{col}", bufs=2)
            nc.vector.stream_shuffle(out=rot[:, :], in_=cur, mask=SHUF_MASK)
            nc.vector.tensor_mul(out=rot[:, :], in0=rot[:, :], in1=sb_sin[:, :])
            nc.vector.tensor_mul(out=cur, in0=cur, in1=sb_cos[:, :])
            adddst = cur if col == 0 else sb_kb[:, t, :]
            nc.vector.tensor_add(out=adddst, in0=cur, in1=rot[:, :])
            sq = wk.tile([128, S], bf16, tag="sq", name=f"sq{t}_{col}", bufs=2)
            if t == 0:
                nc.scalar.activation(
                    out=sq[:, :], in_=adddst, func=AF.Square,
                    accum_out=sb_stat[:, col + t : col + t + 1],
                )
            else:
                nc.vector.tensor_mul(out=sq[:, :], in0=adddst, in1=adddst)
                nc.vector.reduce_sum(
                    out=sb_stat[:, col + t : col + t + 1], in_=sq[:, :],
                    axis=mybir.AxisListType.X,
                )
        # rs_comb = (ssq_q * ssq_k)^-1/2 = exp(-0.5*ln(.)); Ln/Exp share the
        # one loaded ACT table set, and [128,1] activations are ~0.3us
        nc.vector.tensor_mul(
            out=sb_stat[:, 8 + t : 9 + t],
            in0=sb_stat[:, t : t + 1],
            in1=sb_stat[:, 4 + t : 5 + t],
        )
        nc.scalar.activation(
            out=sb_stat[:, 12 + t : 13 + t], in_=sb_stat[:, 8 + t : 9 + t],
            func=AF.Ln, bias=0.0,
        )
        nc.scalar.activation(
            out=sb_stat[:, 16 + t : 17 + t], in_=sb_stat[:, 12 + t : 13 + t],
            func=AF.Exp, scale=-0.5,
        )
        nc.vector.tensor_scalar_mul(
            out=sb_qb[:, t, :], in0=sb_q[:, t, :],
            scalar1=sb_stat[:, 16 + t : 17 + t],
        )

    # ---- attention: head pairs, ACT-saturating pipeline ----
    class PairState:
        def __init__(self, t, pso, av):
            self.t = t
            self.pso = pso
            self.av = av

    def epilogue(st):
        """Drain a pair's av accumulator: denominator row + parks + rcp.

        Mid-attention ACT is exp-saturated so everything rides DVE."""
        t, pso = st.t, st.pso
        dstage = wk.tile([1, 2, S], f32, tag="dst", name=f"dst{t}", bufs=2)
        nc.vector.tensor_copy(out=dstage[:, :, :], in_=pso[64:65, :, :])
        nc.vector.tensor_copy(out=sb_or[0:64, t, :], in_=pso[0:64, 0, :])
        nc.vector.tensor_copy(out=sb_or[64:128, t, :], in_=pso[0:64, 1, :])
        for h in range(2):
            nc.sync.dma_start(
                out=den_sh[:, t, h, :],
                in_=dstage[0:1, h, :].rearrange("o (p c) -> o p c", c=8),
            )
        with nc.allow_low_precision("softmax denominator reciprocal in bf16"):
            nc.vector.reciprocal(out=rcp_sh[:, t, :, :], in_=den_sh[:, t, :, :])
        norm_stage(t)

    def pair(t, fillers=None, prev=None):
        hA, hB = 2 * t, 2 * t + 1
        pss = {}
        ets = {}
        pso = ppo.tile([128, 2, S], f32, tag="ov", name=f"pso{t}")

        def sim(h, jb, beat=True):
            b0 = 64 * (h % 2)
            ps = ppm.tile([128, S], f32, tag="mm", name=f"pss{h}_{jb}")
            pss[(h, jb)] = ps
            if beat and h % 2 < N_BEAT:
                # dense-stream heartbeat: a discarded matmul (overwritten by
                # the real sim's start=True) plugs the per-cycle PE idle gap
                # so HAM never sees an idle window and re-throttles the clock
                nc.tensor.matmul(
                    ps[:, 0:512],
                    lhsT=junk_w[:, 0:128],
                    rhs=junk_w[:, :],
                    start=True,
                    stop=True,
                    skip_group_check=True,
                )
            for nn in range(2):
                nc.tensor.matmul(
                    ps[:, nn * 512 : (nn + 1) * 512],
                    lhsT=sb_kb[b0 : b0 + 64, t, jb * 128 : (jb + 1) * 128],
                    rhs=sb_qb[b0 : b0 + 64, t, nn * 512 : (nn + 1) * 512],
                    start=True,
                    stop=True,
                )

        def expf(h, jb):
            et = wk.tile([128, S], bf16, tag="et", name=f"et{h}_{jb}", bufs=6)
            ets[(h, jb)] = et
            nc.scalar.activation(
                out=et[:, :], in_=pss.pop((h, jb))[:, :], func=AF.Exp, scale=SCALE
            )

        def av(h, jb):
            half = h % 2
            et = ets.pop((h, jb))
            for nn in range(2):
                nc.tensor.matmul(
                    pso[0:65, half, nn * 512 : (nn + 1) * 512],
                    lhsT=sb_vx[:, jb, h * (DH + 1) : (h + 1) * (DH + 1)],
                    rhs=et[:, nn * 512 : (nn + 1) * 512],
                    start=(jb == 0),
                    stop=(jb == 7),
                )

        for jb in range(8):
            has_fill = bool(fillers) and jb in fillers
            if prev is not None and jb < 2:
                # the previous pair's catch-up avs are ready work: put them
                # BEFORE this cycle's sims so the in-order PE queue doesn't
                # idle on the sims' exp wait while they sit behind it
                prev.av(2 * prev.t, 6 + jb)
                prev.av(2 * prev.t + 1, 6 + jb)
                # boundary cycles run PE-light; extra beats into the previous
                # pair's av accumulator (unused partition rows, still live)
                # keep the activity monitor from re-throttling the clock
                for _ in range(2):
                    nc.tensor.matmul(
                        prev.pso[96:128, 0, 0:512],
                        lhsT=junk_w[:, 96:128],
                        rhs=junk_w[:, :],
                        start=True,
                        stop=True,
                        skip_group_check=True,
                        tile_position=(0, 96),
                    )
            sim(hA, jb, beat=not has_fill)
            sim(hB, jb)
            expf(hA, jb)
            expf(hB, jb)
            if jb >= 2:
                av(hA, jb - 2)
                av(hB, jb - 2)
            if prev is not None and jb == 1:
                epilogue(prev)
            if fillers and jb in fillers:
                for f in fillers[jb]:
                    f()
        return PairState(t, pso, av)

    def finish_pair(st):
        """Last pair's drain, pipelined per head: each head's av(7) kicks its
        own dstage->DMA->rcp->rrow chain immediately; parks ride the now-idle
        ACT so the av accumulator frees early for the broadcast tile."""
        t, pso = st.t, st.pso
        hA, hB = 2 * t, 2 * t + 1
        dstage = wk.tile([1, 2, S], f32, tag="dst", name=f"dst{t}", bufs=2)
        st.av(hA, 6)
        st.av(hA, 7)
        nc.vector.tensor_copy(out=dstage[:, 0, :], in_=pso[64:65, 0, :])
        nc.sync.dma_start(
            out=den_sh[:, t, 0, :],
            in_=dstage[0:1, 0, :].rearrange("o (p c) -> o p c", c=8),
        )
        st.av(hB, 6)
        st.av(hB, 7)
        nc.vector.tensor_copy(out=dstage[:, 1, :], in_=pso[64:65, 1, :])
        nc.sync.dma_start(
            out=den_sh[:, t, 1, :],
            in_=dstage[0:1, 1, :].rearrange("o (p c) -> o p c", c=8),
        )
        nc.scalar.copy(out=sb_or[0:64, t, :], in_=pso[0:64, 0, :])
        nc.scalar.copy(out=sb_or[64:128, t, :], in_=pso[0:64, 1, :])
        with nc.allow_low_precision("softmax denominator reciprocal in bf16"):
            for h in range(2):
                nc.vector.reciprocal(out=rcp_sh[:, t, h, :], in_=den_sh[:, t, h, :])
                rr = wk.tile([1, S], bf16, tag="rrow", name=f"rr{t}_{h}", bufs=4)
                rrows[(t, h)] = rr
                nc.sync.dma_start(out=rr[:, :], in_=rcp_sh[:, t, h, :])

    rrows = {}

    def norm_stage(t):
        """DMA the pair's reciprocal rows back to [1, S] bcast operands."""
        for h in range(2):
            rr = wk.tile([1, S], bf16, tag="rrow", name=f"rr{t}_{h}", bufs=4)
            rrows[(t, h)] = rr
            nc.sync.dma_start(out=rr[:, :], in_=rcp_sh[:, t, h, :])

    def norm_mm(t, pool, pre_beats=0):
        """Broadcast 1/den across partitions via K=1 matmuls, then scale.

        The [128, S] broadcast tile borrows a rotation slot from `pool`: for
        pairs 0-2 that's the ppm sim ring (emitted a few cycles into the NEXT
        pair so the PE queue never stalls on the reciprocal roundtrip); for
        pair 3 it's the ppo slot freed by the just-parked av accumulator.
        pre_beats emits keep-warm junk into the tile while the reciprocal
        roundtrip drains (tail only)."""
        pbc = pool.tile([128, S], f32, tag="mm" if pool is ppm else "ov",
                        name=f"pbc{t}")
        for _ in range(pre_beats):
            nc.tensor.matmul(
                pbc[:, 0:512], lhsT=junk_w[:, 0:128], rhs=junk_w[:, :],
                start=True, stop=True, skip_group_check=True,
            )
        for h in range(2):
            base = 64 * h
            rr = rrows.pop((t, h))
            for nn in range(2):
                nc.tensor.matmul(
                    pbc[base : base + 64, nn * 512 : (nn + 1) * 512],
                    lhsT=ones_col[0:1, :],
                    rhs=rr[0:1, nn * 512 : (nn + 1) * 512],
                    start=True,
                    stop=True,
                )
        nc.vector.tensor_mul(out=sb_o[:, t, :], in0=sb_or[:, t, :], in1=pbc[:, :])

    # ---- emission order == scheduling priority ----
    qk_mms(0)
    qk_mms(1)
    qk_evac(0, "act")
    qk_evac(1, "act")
    for jb in range(4):
        v_block(jb, evac="act")
    rope_norm(0)  # t=0 stats on ACT, whole chain on an uncontended DVE
    for jb in range(4, 8):
        v_block(jb, evac="act")
    # keep the PE clock-gate warm across the rope0 tail (PE idle otherwise)
    junk_run(8, "junk1")
    qk_mms(2)
    qk_mms(3)
    qk_evac(2, "dve")
    qk_evac(3, "dve")
    rope_norm(1)

    def qk_filler(b):
        # whole block + evac in one cycle: the tile's ppm-ring residency is
        # what serializes the sims, so keep it as short as possible
        def f():
            qk_mms(b)
            qk_evac(b, "dve")
        return f

    st0 = pair(0, fillers={
        2: [qk_filler(4)],
        5: [qk_filler(5)],
    })
    st1 = pair(1, prev=st0, fillers={
        2: [qk_filler(6)],
        4: [lambda: rope_norm(2)],
        5: [qk_filler(7)],
    })
    st2 = pair(2, prev=st1, fillers={
        2: [lambda: rope_norm(3)],
        4: [lambda: norm_mm(0, ppm)],
        6: [lambda: norm_mm(1, ppm)],
    })
    st3 = pair(3, prev=st2, fillers={
        4: [lambda: norm_mm(2, ppm)],
    })
    finish_pair(st3)

    # ---- output projection tail (all in PSUM) ----
    psf = [ppm.tile([128, S], f32, tag="mm", name=f"psf{ob}") for ob in range(2)]

    def proj_kk(p, obs=(0, 1)):
        for ob in obs:
            for nn in range(2):
                nc.tensor.matmul(
                    psf[ob][:, nn * 512 : (nn + 1) * 512],
                    lhsT=sb_wo[:, p, ob * 128 : (ob + 1) * 128],
                    rhs=sb_o[:, p, nn * 512 : (nn + 1) * 512],
                    start=(p == 0),
                    stop=(p == 3),
                )

    for p in range(3):
        proj_kk(p)
    # pair-3 normalization: broadcast tile from the freed ppo slot
    norm_mm(3, ppo)
    for ob in range(2):
        proj_kk(3, obs=(ob,))
        # bias on both free engines in parallel (ob0 ACT, ob1 DVE)
        if ob == 0:
            nc.scalar.activation(
                out=ot[:, ob, :], in_=psf[ob][:, :], func=AF.Identity,
                bias=sb_bo[:, ob : ob + 1],
            )
        else:
            nc.vector.tensor_scalar_add(
                out=ot[:, ob, :], in0=psf[ob][:, :], scalar1=sb_bo[:, ob : ob + 1]
            )
        nc.sync.dma_start(out=out_d[:, ob, :], in_=ot[:, ob, :])


def _patch_act_tables():
    """Steer the act-table-load pass to one set covering Exp+Copy+Square.

    The default pass picks the first table set containing each activation
    function, which can ping-pong between sets (~2.7us per reload).  Emptying
    every other set forces all activations onto natural_log_exp_and_others ->
    exactly one load.
    """
    import concourse.bacc as bacc

    if getattr(bacc, "_act_tables_patched", False):
        return
    import concourse.hw_specs as hw_specs

    orig = hw_specs.get_activation_tables

    def patched(arch):
        tables = orig(arch)
        keep = "natural_log_exp_and_others"
        assert keep in tables
        return {
            name: (fns if name == keep else set()) for name, fns in tables.items()
        }

    bacc.get_activation_tables = patched
    bacc._act_tables_patched = True


def _build():
    from contextlib import ExitStack

    import concourse.bacc as bacc
    import concourse.tile as tile
    from concourse import mybir

    _patch_act_tables()

    f32 = mybir.dt.float32
    bf16 = mybir.dt.bfloat16
    nc = bacc.Bacc("TRN2", target_bir_lowering=False, debug=False, num_devices=N_CORES)
    aps = (
        nc.dram_tensor("x", [2, 128, 2, S // 2], bf16, kind="ExternalInput").ap(),
        nc.dram_tensor("wqkP", [4, 128, 2, 256], bf16, kind="ExternalInput").ap(),
        nc.dram_tensor("wvT", [128, 2, HID], bf16, kind="ExternalInput").ap(),
        nc.dram_tensor("woT", [128, 4, C], bf16, kind="ExternalInput").ap(),
        nc.dram_tensor("bout", [128, 2], f32, kind="ExternalInput").ap(),
        nc.dram_tensor("cosT", [128, S], bf16, kind="ExternalInput").ap(),
        nc.dram_tensor("sinT", [128, S], bf16, kind="ExternalInput").ap(),
        nc.dram_tensor("out", [128, 2, S], bf16, kind="ExternalOutput").ap(),
    )
    with tile.TileContext(nc) as tc:
        with ExitStack() as ctx:
            _emit(ctx, tc, aps)
    nc.compile()
    return nc


def _get_nc():
    if "nc" not in _CACHE:
        _CACHE["nc"] = _build()
    return _CACHE["nc"]


def _make_in_maps(x, w_qkv, w_out, b_out):
    import ml_dtypes

    bf = ml_dtypes.bfloat16
    xf32 = np.asarray(x, np.float32).reshape(N_CORES, C, S)
    # x shipped s-half-major: [core][s-half][p][kk][512], contiguous per chunk
    x3 = np.ascontiguousarray(
        xf32.reshape(N_CORES, 2, 128, 2, S // 2).transpose(0, 3, 2, 1, 4)
    ).astype(bf)
    wq = np.asarray(w_qkv, np.float32)
    wqT = wq[0:HID, :].T          # [C, 512]
    wkT = wq[HID : 2 * HID, :].T  # [C, 512]
    blocks = []
    for t in range(4):
        blocks.append(wqT[:, 128 * t : 128 * (t + 1)])
        blocks.append(wkT[:, 128 * t : 128 * (t + 1)])
    wqkP = np.concatenate(blocks, axis=1)  # [C, 1024], q0,k0,q1,k1,...
    # shipped o-chunk-major: [chunk][p][kk][256]
    wqk3 = np.ascontiguousarray(
        wqkP.reshape(2, 128, 4, 256).transpose(2, 1, 0, 3)
    ).astype(bf)
    wv3 = np.ascontiguousarray(
        wq[2 * HID : 3 * HID, :].T.reshape(2, 128, HID).transpose(1, 0, 2)
    ).astype(bf)
    wo3 = np.ascontiguousarray(
        np.asarray(w_out, np.float32).T.reshape(4, 128, C).transpose(1, 0, 2)
    ).astype(bf)
    bo = np.ascontiguousarray(np.asarray(b_out, np.float32).reshape(2, 128).T)
    cosT, sinT = _rope_tables()
    shared = {
        "wqkP": wqk3,
        "wvT": wv3,
        "woT": wo3,
        "bout": bo,
        "cosT": cosT.astype(bf),
        "sinT": sinT.astype(bf),
    }
    return [dict(shared, x=np.ascontiguousarray(x3[i])) for i in range(N_CORES)]


def _postprocess(res):
    # out is [128, 2, S] bf16 per core: channel = ob*128 + p
    outs = []
    for r in res.results:
        arr = np.asarray(r["out"], np.float32)  # [128, 2, S]
        outs.append(arr.transpose(1, 0, 2).reshape(C, 32, 32))
    return np.stack(outs, axis=0)


def _run(x, w_qkv, w_out, b_out, trace=False):
    from concourse.bass_utils import run_bass_kernel_spmd

    nc = _get_nc()
    in_maps = _make_in_maps(x, w_qkv, w_out, b_out)
    res = run_bass_kernel_spmd(nc, in_maps, core_ids=list(range(N_CORES)), trace=trace)
    return _postprocess(res), res


def kernel(x, w_qkv, w_out, b_out):
    return _run(x, w_qkv, w_out, b_out, trace=False)[0]
